# revision 32
# baseline (speedup 1.0000x reference)
"""AtomPlacementScheduler Trainium2 kernel.

out[b] = sum_e irfft(rfft(stems[b,e]) * exp(-2i pi f s_be)),  s = sigmoid(TL@W+b)*N.

4-step FFT (N = 32768 = 256 x 128): all heavy work is TensorEngine matmuls.
Host ships, per event, a packed bf16 record [stems | Cre | Cim | Mre | Mim | -Mim]
where C = twiddle*shift-phase (n1 x k2) and M = W1*diag(B) (n1 x k1), so the
device does: stage-1 DFT (2 matmuls), one PSUM->SBUF cast, one complex
elementwise multiply (6 ops split vector/gpsimd), stage-3 DFT (4 matmuls
accumulating X re/im over the 16 events in PSUM).  Per batch, a transpose-free
inverse FFT: chunked matmuls produce G in transposed layout directly, twiddle,
then the final inner inverse DFT with real-part-only output.

Pure data parallel over batch: 64 batches / 8 cores = 8 per core.
Self-contained: hardcodes shapes B=64, E=16, N=32768, n_cores=8.
"""
import numpy as np
import ml_dtypes

N = 32768
N1 = 128   # outer DFT size (n1, k1)
N2 = 256   # inner DFT size (n2, k2)
E = 16
B = 64
NCORES = 8
BC = B // NCORES      # 8 batches per core
S = BC * E            # 128 signals per core
K1 = 65               # k1 = 0..64 covers k = k2 + 256*k1 up to Nyquist
# 256 stems | 256 Cre | 256 -Cim | 256 Cim | 256 Cre | 65 Mre | 65 Mim |
# 65 -Mim | 1 pad.  The C block [Cre|-Cim|Cim|Cre] lets one FD-1024 multiply
# (p1sb broadcast-read twice) produce all four products with signs arranged so
# both combines are adds, done as one block-strided FD-512 add.
RECW = 1476

F32 = np.float32
BF16 = ml_dtypes.bfloat16


def _host_consts():
    n1 = np.arange(N1)
    n2 = np.arange(N2)
    k2 = np.arange(N2)
    k1 = np.arange(K1)
    W2 = np.exp(-2j * np.pi * np.outer(n2, k2) / N2)            # (n2, k2)
    W2cat = np.concatenate([W2.real, W2.imag], 1)               # (256, 512)
    E1 = np.exp(+2j * np.pi * np.outer(k1[:64], n1) / N1)       # (k1<64, m)
    e1cat = np.zeros((K1, 384))
    e1cat[:64, 0:128] = E1.real
    e1cat[:64, 128:256] = E1.imag
    e1cat[:64, 256:384] = -E1.imag
    TinvT = np.exp(+2j * np.pi * np.outer(k2, n1) / N)          # (k2, m)
    tinv = np.zeros((2, 128, 256))
    for c in range(2):
        tinv[c, :, 0:128] = TinvT.real[c * 128:(c + 1) * 128]
        tinv[c, :, 128:256] = TinvT.imag[c * 128:(c + 1) * 128]
    E2 = np.exp(+2j * np.pi * np.outer(k2, n2) / N2) * (2.0 / N)  # (k2, n2)
    e2 = np.zeros((2, 128, 512))
    for c in range(2):
        e2[c, :, 0:256] = E2.real[c * 128:(c + 1) * 128]
        e2[c, :, 256:512] = -E2.imag[c * 128:(c + 1) * 128]
    return W2cat, e1cat, tinv, e2


def _build_graph():
    import concourse.bass as bass
    import concourse.mybir as mybir
    import concourse.tile as tile
    from concourse import bacc

    dt = mybir.dt
    nc = bacc.Bacc("TRN2", target_bir_lowering=False, debug=False, num_devices=NCORES)

    rec_d = nc.dram_tensor("rec", [BC, E, 128, RECW], dt.bfloat16, kind="ExternalInput")
    w2_d = nc.dram_tensor("w2cat", [N2, 512], dt.bfloat16, kind="ExternalInput")
    e1_d = nc.dram_tensor("e1cat", [K1, 384], dt.bfloat16, kind="ExternalInput")
    tinv_d = nc.dram_tensor("tinv", [2, 128, 256], dt.bfloat16, kind="ExternalInput")
    e2_d = nc.dram_tensor("e2", [2, 128, 512], dt.bfloat16, kind="ExternalInput")
    out_d = nc.dram_tensor("out", [BC, N2, N1], dt.float32, kind="ExternalOutput")
    aux_d = nc.dram_tensor("aux", [BC, 2], dt.float32, kind="ExternalOutput")

    LAG = 3

    with tile.TileContext(nc) as tc:
        with (
            tc.tile_pool(name="const", bufs=1) as cpool,
            tc.tile_pool(name="rec", bufs=LAG + 3) as recpool,
            tc.tile_pool(name="work", bufs=6) as pool,
            tc.tile_pool(name="inv", bufs=2) as ipool,
            tc.tile_pool(name="p1p", bufs=3, space="PSUM") as p1pool,
            tc.tile_pool(name="pxp", bufs=2, space="PSUM") as pxpool,
            tc.tile_pool(name="pgp", bufs=2, space="PSUM") as pgpool,
            tc.tile_pool(name="pyp", bufs=1, space="PSUM") as pypool,
        ):
            w2_0 = cpool.tile([128, 512], dt.bfloat16, tag="w2_0")
            w2_1 = cpool.tile([128, 512], dt.bfloat16, tag="w2_1")
            nc.sync.dma_start(w2_0[:], w2_d[0:128, :])
            nc.sync.dma_start(w2_1[:], w2_d[128:256, :])
            e1 = cpool.tile([K1, 384], dt.bfloat16, tag="e1")
            nc.sync.dma_start(e1[:], e1_d[:])
            tinv_0 = cpool.tile([128, 256], dt.bfloat16, tag="tinv0")
            tinv_1 = cpool.tile([128, 256], dt.bfloat16, tag="tinv1")
            nc.sync.dma_start(tinv_0[:], tinv_d[0])
            nc.sync.dma_start(tinv_1[:], tinv_d[1])
            e2_0 = cpool.tile([128, 512], dt.bfloat16, tag="e2_0")
            e2_1 = cpool.tile([128, 512], dt.bfloat16, tag="e2_1")
            nc.sync.dma_start(e2_0[:], e2_d[0])
            nc.sync.dma_start(e2_1[:], e2_d[1])
            tinv = [tinv_0, tinv_1]
            e2t = [e2_0, e2_1]
            w2 = [w2_0, w2_1]

            slots = {}

            def front(i):
                b, e = divmod(i, E)
                rec = recpool.tile([128, RECW], dt.bfloat16, tag="rec")
                nc.sync.dma_start(rec[:], rec_d[b, e])
                p1 = p1pool.tile([128, 512], dt.float32, tag="p1")
                nc.tensor.matmul(p1[:], rec[:, 0:128], w2[0][:], start=True, stop=False)
                nc.tensor.matmul(p1[:], rec[:, 128:256], w2[1][:], start=False, stop=True)
                slots[i] = (rec, p1)

            def back(i):
                b, e = divmod(i, E)
                rec, p1 = slots.pop(i)
                p1sb = pool.tile([128, 512], dt.bfloat16, tag="p1sb")
                nc.scalar.copy(p1sb[:], p1[:])
                m12 = pool.tile([128, 1024], dt.bfloat16, tag="m12")
                uv = pool.tile([128, 512], dt.bfloat16, tag="uv")
                # m12 = [p1re*Cre | p1im*(-Cim) | p1re*Cim | p1im*Cre]:
                # one FD-1024 multiply reading p1sb twice via a 0-stride dim.
                p1rep = p1sb[:].unsqueeze(1).broadcast_to([128, 2, 512])
                nc.vector.tensor_mul(
                    m12[:].rearrange("p (a b) -> p a b", a=2),
                    p1rep, rec[:, 256:1280].rearrange("p (a b) -> p a b", a=2))
                # uv = [Ure | Uim] = pairwise block add of m12, on gpsimd so
                # vector only does the one big multiply per event.
                m12v = m12[:].rearrange("p (a b) -> p a b", a=2)
                nc.gpsimd.tensor_add(
                    uv[:].rearrange("p (a b) -> p a b", a=2),
                    m12v[:, :, 0:256], m12v[:, :, 256:512])
                if e == 0:
                    slots[("pX", b)] = pxpool.tile([K1, 512], dt.float32, tag="pX",
                                                   name="pX")
                pX = slots[("pX", b)]
                # One accumulation group per PSUM bank: start only on the very
                # first matmul (start marks the whole 2KB zero region), stop on
                # the very last.
                st = e == 0
                sp = e == E - 1
                nc.tensor.matmul(pX[:, 0:256], rec[:, 1280:1345], uv[:, 0:256],
                                 start=st, stop=False)
                nc.tensor.matmul(pX[:, 0:256], rec[:, 1410:1475], uv[:, 256:512],
                                 start=False, stop=False)
                nc.tensor.matmul(pX[:, 256:512], rec[:, 1280:1345], uv[:, 256:512],
                                 start=False, stop=False)
                nc.tensor.matmul(pX[:, 256:512], rec[:, 1345:1410], uv[:, 0:256],
                                 start=False, stop=sp)
                if e == E - 1:
                    inverse(i, b, slots.pop(("pX", b)))

            pending = {}

            def sched(idx, fn):
                pending.setdefault(idx, []).append(fn)

            def inverse(i, b, pX):
                # The inverse is issued in stages deferred across the next
                # events so each instruction reaches its engine queue with
                # inputs already computed (no head-of-line blocking).
                xsb = ipool.tile([K1, 512], dt.bfloat16, tag="xsb")
                pG = pgpool.tile([128, 512], dt.float32, tag="pG", name="pG")
                pY = pypool.tile([128, 512], dt.float32, tag="pY", name="pY")

                def st1():
                    nc.scalar.copy(xsb[:], pX[:])
                    nc.gpsimd.dma_start(aux_d[b, 0:1], xsb[0:1, 0:1])
                    nc.gpsimd.dma_start(aux_d[b, 1:2], xsb[64:65, 0:1])

                def st2():
                    for c in range(2):
                        xre = xsb[:, c * 128:(c + 1) * 128]
                        xim = xsb[:, 256 + c * 128:256 + (c + 1) * 128]
                        o = c * 256
                        nc.tensor.matmul(pG[:, o:o + 128], xre, e1[:, 0:128],
                                         start=(c == 0), stop=False)
                        nc.tensor.matmul(pG[:, o:o + 128], xim, e1[:, 256:384],
                                         start=False, stop=False)
                        nc.tensor.matmul(pG[:, o + 128:o + 256], xre,
                                         e1[:, 128:256], start=False, stop=False)
                        nc.tensor.matmul(pG[:, o + 128:o + 256], xim,
                                         e1[:, 0:128], start=False, stop=(c == 1))

                gts = []

                def st3():
                    for c in range(2):
                        gsb = ipool.tile([128, 256], dt.bfloat16, tag=f"gsb{c}")
                        nc.scalar.copy(gsb[:], pG[:, c * 256:(c + 1) * 256])
                        g1 = ipool.tile([128, 128], dt.bfloat16, tag=f"g1{c}")
                        g2 = ipool.tile([128, 128], dt.bfloat16, tag=f"g2{c}")
                        g3 = ipool.tile([128, 128], dt.bfloat16, tag=f"g3{c}")
                        g4 = ipool.tile([128, 128], dt.bfloat16, tag=f"g4{c}")
                        gt = ipool.tile([128, 256], dt.bfloat16, tag=f"gt{c}")
                        nc.vector.tensor_mul(g1[:], gsb[:, 0:128],
                                             tinv[c][:, 0:128])
                        nc.vector.tensor_mul(g2[:], gsb[:, 128:256],
                                             tinv[c][:, 128:256])
                        nc.vector.tensor_sub(gt[:, 0:128], g1[:], g2[:])
                        nc.gpsimd.tensor_mul(g3[:], gsb[:, 0:128],
                                             tinv[c][:, 128:256])
                        nc.gpsimd.tensor_mul(g4[:], gsb[:, 128:256],
                                             tinv[c][:, 0:128])
                        nc.vector.tensor_add(gt[:, 128:256], g3[:], g4[:])
                        gts.append(gt)

                def st4():
                    for j in range(2):
                        nc.tensor.matmul(pY[:, j * 128:(j + 1) * 128],
                                         e2t[0][:, j * 128:(j + 1) * 128],
                                         gts[0][:, 0:128], start=(j == 0),
                                         stop=False)
                        nc.tensor.matmul(pY[:, j * 128:(j + 1) * 128],
                                         e2t[0][:, 256 + j * 128:256 + (j + 1) * 128],
                                         gts[0][:, 128:256], start=False,
                                         stop=False)
                        nc.tensor.matmul(pY[:, j * 128:(j + 1) * 128],
                                         e2t[1][:, j * 128:(j + 1) * 128],
                                         gts[1][:, 0:128], start=False,
                                         stop=False)
                        nc.tensor.matmul(pY[:, j * 128:(j + 1) * 128],
                                         e2t[1][:, 256 + j * 128:256 + (j + 1) * 128],
                                         gts[1][:, 128:256], start=False,
                                         stop=(j == 1))

                def st5():
                    for j in range(2):
                        ysb = ipool.tile([128, 128], dt.float32, tag=f"ysb{j}")
                        nc.scalar.copy(ysb[:], pY[:, j * 128:(j + 1) * 128])
                        nc.sync.dma_start(out_d[b, j * 128:(j + 1) * 128, :],
                                          ysb[:])

                sched(i, st1)
                sched(i, st2)
                sched(i, st3)
                sched(i, st4)
                sched(i, st5)

            for i in range(S + LAG + 10):
                if i < S:
                    front(i)
                j = i - LAG
                if 0 <= j < S:
                    back(j)
                for fn in pending.pop(j, ()):
                    fn()

    nc.compile()
    return nc


def kernel(time_latent, stems, targets, W_pos, b_pos):
    from concourse.bass_utils import run_bass_kernel_spmd

    # host: positions (tiny linear+sigmoid, fp32 exactly like the reference)
    z = np.einsum("bed,od->beo", time_latent.astype(F32), W_pos.astype(F32))
    z = z.reshape(B, E) + b_pos.reshape(1)[0]
    pos = 1.0 / (1.0 + np.exp(-z, dtype=F32))
    s = (pos * np.float32(N)).astype(np.float64)

    W2cat, e1cat, tinv, e2 = _host_consts()
    n1 = np.arange(N1)
    k2 = np.arange(N2)
    k1 = np.arange(K1)
    T = np.exp(-2j * np.pi * np.outer(n1, k2) / N)   # (n1, k2)
    W1 = np.exp(-2j * np.pi * np.outer(n1, k1) / N1)  # (n1, k1)

    w2cat_b = W2cat.astype(BF16)
    e1cat_b = e1cat.astype(BF16)
    tinv_b = tinv.astype(BF16)
    e2_b = e2.astype(BF16)

    nc = _build_graph()
    in_maps = []
    for c in range(NCORES):
        sl = slice(c * BC, (c + 1) * BC)
        s_flat = s[sl].reshape(-1)                                   # (S,)
        rec = np.empty((S, 128, RECW), dtype=BF16)
        # stems: (S, 256, 128) -> (S, 2, 128, 128) -> (S, 128, 2, 128)
        st = stems[sl].reshape(S, 2, 128, 128).transpose(0, 2, 1, 3)
        rec[:, :, 0:256] = st.reshape(S, 128, 256).astype(BF16)
        A = np.exp(-2j * np.pi * np.outer(s_flat, k2) / N)           # (S, k2)
        C = T[None, :, :] * A[:, None, :]                            # (S, n1, k2)
        cre = C.real.astype(BF16)
        cim = C.imag.astype(BF16)
        rec[:, :, 256:512] = cre
        rec[:, :, 512:768] = -cim
        rec[:, :, 768:1024] = cim
        rec[:, :, 1024:1280] = cre
        del C, cre, cim
        Bt = np.exp(-2j * np.pi * np.outer(s_flat, k1) / N1)         # (S, k1)
        M = W1[None, :, :] * Bt[:, None, :]                          # (S, n1, k1)
        rec[:, :, 1280:1345] = M.real.astype(BF16)
        rec[:, :, 1345:1410] = M.imag.astype(BF16)
        rec[:, :, 1410:1475] = (-M.imag).astype(BF16)
        rec[:, :, 1475:1476] = 0
        del M
        in_maps.append({
            "rec": rec.reshape(BC, E, 128, RECW),
            "w2cat": w2cat_b,
            "e1cat": e1cat_b,
            "tinv": tinv_b,
            "e2": e2_b,
        })

    import os
    trace = bool(int(os.environ.get("ATHENA_TRACE", "0")))
    res = run_bass_kernel_spmd(nc, in_maps, core_ids=list(range(NCORES)), trace=trace)
    if trace:
        print(f"HW exec time: {res.exec_time_ns} ns")
    outs = []
    sign = np.where(np.arange(N) % 2 == 0, 1.0, -1.0).astype(F32)
    for c in range(NCORES):
        y = res.results[c]["out"].reshape(BC, N).astype(F32)
        aux = res.results[c]["aux"].astype(F32)          # (BC, 2) = X0, XNyq
        y = y + (-aux[:, 0:1] + sign[None, :] * aux[:, 1:2]) / np.float32(N)
        outs.append(y)
    return np.concatenate(outs, 0).reshape(B, 1, N).astype(F32)


# revision 33
# speedup vs baseline: 1.1837x; 1.1837x over previous
"""AtomPlacementScheduler Trainium2 kernel.

out[b] = sum_e irfft(rfft(stems[b,e]) * exp(-2i pi f s_be)),  s = sigmoid(TL@W+b)*N.

4-step FFT (N = 32768 = 256 x 128): all heavy work is TensorEngine matmuls.
Host ships, per event, a packed bf16 record [stems | Cre | Cim | Mre | Mim | -Mim]
where C = twiddle*shift-phase (n1 x k2) and M = W1*diag(B) (n1 x k1), so the
device does: stage-1 DFT (2 matmuls), one PSUM->SBUF cast, one complex
elementwise multiply (6 ops split vector/gpsimd), stage-3 DFT (4 matmuls
accumulating X re/im over the 16 events in PSUM).  Per batch, a transpose-free
inverse FFT: chunked matmuls produce G in transposed layout directly, twiddle,
then the final inner inverse DFT with real-part-only output.

Pure data parallel over batch: 64 batches / 8 cores = 8 per core.
Self-contained: hardcodes shapes B=64, E=16, N=32768, n_cores=8.
"""
import numpy as np
import ml_dtypes

N = 32768
N1 = 128   # outer DFT size (n1, k1)
N2 = 256   # inner DFT size (n2, k2)
E = 16
B = 64
NCORES = 8
BC = B // NCORES      # 8 batches per core
S = BC * E            # 128 signals per core
K1 = 65               # k1 = 0..64 covers k = k2 + 256*k1 up to Nyquist
# 256 stems | 256 Cre | 256 -Cim | 256 Cim | 256 Cre | 65 Mre | 65 Mim |
# 65 -Mim | 1 pad.  The C block [Cre|-Cim|Cim|Cre] lets one FD-1024 multiply
# (p1sb broadcast-read twice) produce all four products with signs arranged so
# both combines are adds, done as one block-strided FD-512 add.
RECW = 1476

F32 = np.float32
BF16 = ml_dtypes.bfloat16


def _host_consts():
    n1 = np.arange(N1)
    n2 = np.arange(N2)
    k2 = np.arange(N2)
    k1 = np.arange(K1)
    W2 = np.exp(-2j * np.pi * np.outer(n2, k2) / N2)            # (n2, k2)
    W2cat = np.concatenate([W2.real, W2.imag], 1)               # (256, 512)
    E1 = np.exp(+2j * np.pi * np.outer(k1[:64], n1) / N1)       # (k1<64, m)
    e1cat = np.zeros((K1, 384))
    e1cat[:64, 0:128] = E1.real
    e1cat[:64, 128:256] = E1.imag
    e1cat[:64, 256:384] = -E1.imag
    TinvT = np.exp(+2j * np.pi * np.outer(k2, n1) / N)          # (k2, m)
    tinv = np.zeros((2, 128, 256))
    for c in range(2):
        tinv[c, :, 0:128] = TinvT.real[c * 128:(c + 1) * 128]
        tinv[c, :, 128:256] = TinvT.imag[c * 128:(c + 1) * 128]
    E2 = np.exp(+2j * np.pi * np.outer(k2, n2) / N2) * (2.0 / N)  # (k2, n2)
    e2 = np.zeros((2, 128, 512))
    for c in range(2):
        e2[c, :, 0:256] = E2.real[c * 128:(c + 1) * 128]
        e2[c, :, 256:512] = -E2.imag[c * 128:(c + 1) * 128]
    return W2cat, e1cat, tinv, e2


def _build_graph():
    import concourse.bass as bass
    import concourse.mybir as mybir
    import concourse.tile as tile
    from concourse import bacc

    dt = mybir.dt
    nc = bacc.Bacc("TRN2", target_bir_lowering=False, debug=False, num_devices=NCORES)

    rec_d = nc.dram_tensor("rec", [BC, E, 128, RECW], dt.bfloat16, kind="ExternalInput")
    w2_d = nc.dram_tensor("w2cat", [N2, 512], dt.bfloat16, kind="ExternalInput")
    e1_d = nc.dram_tensor("e1cat", [K1, 384], dt.bfloat16, kind="ExternalInput")
    tinv_d = nc.dram_tensor("tinv", [2, 128, 256], dt.bfloat16, kind="ExternalInput")
    e2_d = nc.dram_tensor("e2", [2, 128, 512], dt.bfloat16, kind="ExternalInput")
    out_d = nc.dram_tensor("out", [BC, N2, N1], dt.float32, kind="ExternalOutput")
    aux_d = nc.dram_tensor("aux", [BC, 2], dt.float32, kind="ExternalOutput")

    LAG = 3

    with tile.TileContext(nc) as tc:
        with (
            tc.tile_pool(name="const", bufs=1) as cpool,
            tc.tile_pool(name="rec", bufs=LAG + 3) as recpool,
            tc.tile_pool(name="work", bufs=6) as pool,
            tc.tile_pool(name="inv", bufs=2) as ipool,
            tc.tile_pool(name="p1p", bufs=3, space="PSUM") as p1pool,
            tc.tile_pool(name="pxp", bufs=2, space="PSUM") as pxpool,
            tc.tile_pool(name="pgp", bufs=2, space="PSUM") as pgpool,
            tc.tile_pool(name="pyp", bufs=1, space="PSUM") as pypool,
        ):
            w2_0 = cpool.tile([128, 512], dt.bfloat16, tag="w2_0")
            w2_1 = cpool.tile([128, 512], dt.bfloat16, tag="w2_1")
            nc.sync.dma_start(w2_0[:], w2_d[0:128, :])
            nc.sync.dma_start(w2_1[:], w2_d[128:256, :])
            e1 = cpool.tile([K1, 384], dt.bfloat16, tag="e1")
            nc.sync.dma_start(e1[:], e1_d[:])
            tinv_0 = cpool.tile([128, 256], dt.bfloat16, tag="tinv0")
            tinv_1 = cpool.tile([128, 256], dt.bfloat16, tag="tinv1")
            nc.sync.dma_start(tinv_0[:], tinv_d[0])
            nc.sync.dma_start(tinv_1[:], tinv_d[1])
            e2_0 = cpool.tile([128, 512], dt.bfloat16, tag="e2_0")
            e2_1 = cpool.tile([128, 512], dt.bfloat16, tag="e2_1")
            nc.sync.dma_start(e2_0[:], e2_d[0])
            nc.sync.dma_start(e2_1[:], e2_d[1])
            tinv = [tinv_0, tinv_1]
            e2t = [e2_0, e2_1]
            w2 = [w2_0, w2_1]

            slots = {}

            def front(i):
                b, e = divmod(i, E)
                rec = recpool.tile([128, RECW], dt.bfloat16, tag="rec")
                nc.sync.dma_start(rec[:], rec_d[b, e])
                p1 = p1pool.tile([128, 512], dt.float32, tag="p1")
                nc.tensor.matmul(p1[:], rec[:, 0:128], w2[0][:], start=True, stop=False)
                nc.tensor.matmul(p1[:], rec[:, 128:256], w2[1][:], start=False, stop=True)
                slots[i] = (rec, p1)

            def back(i):
                b, e = divmod(i, E)
                rec, p1 = slots.pop(i)
                p1sb = pool.tile([128, 512], dt.bfloat16, tag="p1sb")
                nc.scalar.copy(p1sb[:], p1[:])
                m12 = pool.tile([128, 1024], dt.bfloat16, tag="m12")
                uv = pool.tile([128, 512], dt.bfloat16, tag="uv")
                # m12 = [p1re*Cre | p1im*(-Cim) | p1re*Cim | p1im*Cre]:
                # one FD-1024 multiply reading p1sb twice via a 0-stride dim.
                p1rep = p1sb[:].unsqueeze(1).broadcast_to([128, 2, 512])
                nc.vector.tensor_mul(
                    m12[:].rearrange("p (a b) -> p a b", a=2),
                    p1rep, rec[:, 256:1280].rearrange("p (a b) -> p a b", a=2))
                # uv = [Ure | Uim] = pairwise block add of m12.
                m12v = m12[:].rearrange("p (a b) -> p a b", a=2)
                nc.vector.tensor_add(
                    uv[:].rearrange("p (a b) -> p a b", a=2),
                    m12v[:, :, 0:256], m12v[:, :, 256:512])
                if e == 0:
                    slots[("pX", b)] = pxpool.tile([K1, 512], dt.float32, tag="pX",
                                                   name="pX")
                pX = slots[("pX", b)]
                # One accumulation group per PSUM bank: start only on the very
                # first matmul (start marks the whole 2KB zero region), stop on
                # the very last.
                st = e == 0
                sp = e == E - 1
                nc.tensor.matmul(pX[:, 0:256], rec[:, 1280:1345], uv[:, 0:256],
                                 start=st, stop=False)
                nc.tensor.matmul(pX[:, 0:256], rec[:, 1410:1475], uv[:, 256:512],
                                 start=False, stop=False)
                nc.tensor.matmul(pX[:, 256:512], rec[:, 1280:1345], uv[:, 256:512],
                                 start=False, stop=False)
                nc.tensor.matmul(pX[:, 256:512], rec[:, 1345:1410], uv[:, 0:256],
                                 start=False, stop=sp)
                if e == E - 1:
                    inverse(i, b, slots.pop(("pX", b)))

            pending = {}

            def sched(idx, fn):
                pending.setdefault(idx, []).append(fn)

            def inverse(i, b, pX):
                # The inverse is issued in stages deferred across the next
                # events so each instruction reaches its engine queue with
                # inputs already computed (no head-of-line blocking).
                xsb = ipool.tile([K1, 512], dt.bfloat16, tag="xsb")
                pG = pgpool.tile([128, 512], dt.float32, tag="pG", name="pG")
                pY = pypool.tile([128, 512], dt.float32, tag="pY", name="pY")

                def st1():
                    nc.scalar.copy(xsb[:], pX[:])
                    nc.gpsimd.dma_start(aux_d[b, 0:1], xsb[0:1, 0:1])
                    nc.gpsimd.dma_start(aux_d[b, 1:2], xsb[64:65, 0:1])

                def st2():
                    for c in range(2):
                        xre = xsb[:, c * 128:(c + 1) * 128]
                        xim = xsb[:, 256 + c * 128:256 + (c + 1) * 128]
                        o = c * 256
                        nc.tensor.matmul(pG[:, o:o + 128], xre, e1[:, 0:128],
                                         start=(c == 0), stop=False)
                        nc.tensor.matmul(pG[:, o:o + 128], xim, e1[:, 256:384],
                                         start=False, stop=False)
                        nc.tensor.matmul(pG[:, o + 128:o + 256], xre,
                                         e1[:, 128:256], start=False, stop=False)
                        nc.tensor.matmul(pG[:, o + 128:o + 256], xim,
                                         e1[:, 0:128], start=False, stop=(c == 1))

                gts = []

                def st3():
                    for c in range(2):
                        gsb = ipool.tile([128, 256], dt.bfloat16, tag=f"gsb{c}")
                        nc.scalar.copy(gsb[:], pG[:, c * 256:(c + 1) * 256])
                        g1 = ipool.tile([128, 128], dt.bfloat16, tag=f"g1{c}")
                        g2 = ipool.tile([128, 128], dt.bfloat16, tag=f"g2{c}")
                        g3 = ipool.tile([128, 128], dt.bfloat16, tag=f"g3{c}")
                        g4 = ipool.tile([128, 128], dt.bfloat16, tag=f"g4{c}")
                        gt = ipool.tile([128, 256], dt.bfloat16, tag=f"gt{c}")
                        nc.vector.tensor_mul(g1[:], gsb[:, 0:128],
                                             tinv[c][:, 0:128])
                        nc.vector.tensor_mul(g2[:], gsb[:, 128:256],
                                             tinv[c][:, 128:256])
                        nc.vector.tensor_sub(gt[:, 0:128], g1[:], g2[:])
                        nc.gpsimd.tensor_mul(g3[:], gsb[:, 0:128],
                                             tinv[c][:, 128:256])
                        nc.gpsimd.tensor_mul(g4[:], gsb[:, 128:256],
                                             tinv[c][:, 0:128])
                        nc.vector.tensor_add(gt[:, 128:256], g3[:], g4[:])
                        gts.append(gt)

                def st4():
                    for j in range(2):
                        nc.tensor.matmul(pY[:, j * 128:(j + 1) * 128],
                                         e2t[0][:, j * 128:(j + 1) * 128],
                                         gts[0][:, 0:128], start=(j == 0),
                                         stop=False)
                        nc.tensor.matmul(pY[:, j * 128:(j + 1) * 128],
                                         e2t[0][:, 256 + j * 128:256 + (j + 1) * 128],
                                         gts[0][:, 128:256], start=False,
                                         stop=False)
                        nc.tensor.matmul(pY[:, j * 128:(j + 1) * 128],
                                         e2t[1][:, j * 128:(j + 1) * 128],
                                         gts[1][:, 0:128], start=False,
                                         stop=False)
                        nc.tensor.matmul(pY[:, j * 128:(j + 1) * 128],
                                         e2t[1][:, 256 + j * 128:256 + (j + 1) * 128],
                                         gts[1][:, 128:256], start=False,
                                         stop=(j == 1))

                def st5():
                    for j in range(2):
                        ysb = ipool.tile([128, 128], dt.float32, tag=f"ysb{j}")
                        nc.scalar.copy(ysb[:], pY[:, j * 128:(j + 1) * 128])
                        nc.sync.dma_start(out_d[b, j * 128:(j + 1) * 128, :],
                                          ysb[:])

                sched(i, st1)
                sched(i, st2)
                sched(i, st3)
                sched(i, st4)
                sched(i, st5)

            for i in range(S + LAG + 10):
                if i < S:
                    front(i)
                j = i - LAG
                if 0 <= j < S:
                    back(j)
                for fn in pending.pop(j, ()):
                    fn()

    nc.compile()
    return nc


def kernel(time_latent, stems, targets, W_pos, b_pos):
    from concourse.bass_utils import run_bass_kernel_spmd

    # host: positions (tiny linear+sigmoid, fp32 exactly like the reference)
    z = np.einsum("bed,od->beo", time_latent.astype(F32), W_pos.astype(F32))
    z = z.reshape(B, E) + b_pos.reshape(1)[0]
    pos = 1.0 / (1.0 + np.exp(-z, dtype=F32))
    s = (pos * np.float32(N)).astype(np.float64)

    W2cat, e1cat, tinv, e2 = _host_consts()
    n1 = np.arange(N1)
    k2 = np.arange(N2)
    k1 = np.arange(K1)
    T = np.exp(-2j * np.pi * np.outer(n1, k2) / N)   # (n1, k2)
    W1 = np.exp(-2j * np.pi * np.outer(n1, k1) / N1)  # (n1, k1)

    w2cat_b = W2cat.astype(BF16)
    e1cat_b = e1cat.astype(BF16)
    tinv_b = tinv.astype(BF16)
    e2_b = e2.astype(BF16)

    nc = _build_graph()
    in_maps = []
    for c in range(NCORES):
        sl = slice(c * BC, (c + 1) * BC)
        s_flat = s[sl].reshape(-1)                                   # (S,)
        rec = np.empty((S, 128, RECW), dtype=BF16)
        # stems: (S, 256, 128) -> (S, 2, 128, 128) -> (S, 128, 2, 128)
        st = stems[sl].reshape(S, 2, 128, 128).transpose(0, 2, 1, 3)
        rec[:, :, 0:256] = st.reshape(S, 128, 256).astype(BF16)
        A = np.exp(-2j * np.pi * np.outer(s_flat, k2) / N)           # (S, k2)
        C = T[None, :, :] * A[:, None, :]                            # (S, n1, k2)
        cre = C.real.astype(BF16)
        cim = C.imag.astype(BF16)
        rec[:, :, 256:512] = cre
        rec[:, :, 512:768] = -cim
        rec[:, :, 768:1024] = cim
        rec[:, :, 1024:1280] = cre
        del C, cre, cim
        Bt = np.exp(-2j * np.pi * np.outer(s_flat, k1) / N1)         # (S, k1)
        M = W1[None, :, :] * Bt[:, None, :]                          # (S, n1, k1)
        rec[:, :, 1280:1345] = M.real.astype(BF16)
        rec[:, :, 1345:1410] = M.imag.astype(BF16)
        rec[:, :, 1410:1475] = (-M.imag).astype(BF16)
        rec[:, :, 1475:1476] = 0
        del M
        in_maps.append({
            "rec": rec.reshape(BC, E, 128, RECW),
            "w2cat": w2cat_b,
            "e1cat": e1cat_b,
            "tinv": tinv_b,
            "e2": e2_b,
        })

    import os
    trace = bool(int(os.environ.get("ATHENA_TRACE", "0")))
    res = run_bass_kernel_spmd(nc, in_maps, core_ids=list(range(NCORES)), trace=trace)
    if trace:
        print(f"HW exec time: {res.exec_time_ns} ns")
    outs = []
    sign = np.where(np.arange(N) % 2 == 0, 1.0, -1.0).astype(F32)
    for c in range(NCORES):
        y = res.results[c]["out"].reshape(BC, N).astype(F32)
        aux = res.results[c]["aux"].astype(F32)          # (BC, 2) = X0, XNyq
        y = y + (-aux[:, 0:1] + sign[None, :] * aux[:, 1:2]) / np.float32(N)
        outs.append(y)
    return np.concatenate(outs, 0).reshape(B, 1, N).astype(F32)


# revision 34
# speedup vs baseline: 1.2249x; 1.0348x over previous
"""AtomPlacementScheduler Trainium2 kernel.

out[b] = sum_e irfft(rfft(stems[b,e]) * exp(-2i pi f s_be)),  s = sigmoid(TL@W+b)*N.

4-step FFT (N = 32768 = 256 x 128): all heavy work is TensorEngine matmuls.
Host ships, per event, a packed bf16 record [stems | Cre | Cim | Mre | Mim | -Mim]
where C = twiddle*shift-phase (n1 x k2) and M = W1*diag(B) (n1 x k1), so the
device does: stage-1 DFT (2 matmuls), one PSUM->SBUF cast, one complex
elementwise multiply (6 ops split vector/gpsimd), stage-3 DFT (4 matmuls
accumulating X re/im over the 16 events in PSUM).  Per batch, a transpose-free
inverse FFT: chunked matmuls produce G in transposed layout directly, twiddle,
then the final inner inverse DFT with real-part-only output.

Pure data parallel over batch: 64 batches / 8 cores = 8 per core.
Self-contained: hardcodes shapes B=64, E=16, N=32768, n_cores=8.
"""
import numpy as np
import ml_dtypes

N = 32768
N1 = 128   # outer DFT size (n1, k1)
N2 = 256   # inner DFT size (n2, k2)
E = 16
B = 64
NCORES = 8
BC = B // NCORES      # 8 batches per core
S = BC * E            # 128 signals per core
K1 = 65               # k1 = 0..64 covers k = k2 + 256*k1 up to Nyquist
# 256 stems | 256 Cre | 256 -Cim | 256 Cim | 256 Cre | 65 Mre | 65 Mim |
# 65 -Mim | 1 pad.  The C block [Cre|-Cim|Cim|Cre] lets one FD-1024 multiply
# (p1sb broadcast-read twice) produce all four products with signs arranged so
# both combines are adds, done as one block-strided FD-512 add.
RECW = 1476

F32 = np.float32
BF16 = ml_dtypes.bfloat16


def _host_consts():
    n1 = np.arange(N1)
    n2 = np.arange(N2)
    k2 = np.arange(N2)
    k1 = np.arange(K1)
    W2 = np.exp(-2j * np.pi * np.outer(n2, k2) / N2)            # (n2, k2)
    W2cat = np.concatenate([W2.real, W2.imag], 1)               # (256, 512)
    E1 = np.exp(+2j * np.pi * np.outer(k1[:64], n1) / N1)       # (k1<64, m)
    e1cat = np.zeros((K1, 384))
    e1cat[:64, 0:128] = E1.real
    e1cat[:64, 128:256] = E1.imag
    e1cat[:64, 256:384] = -E1.imag
    TinvT = np.exp(+2j * np.pi * np.outer(k2, n1) / N)          # (k2, m)
    tinv = np.zeros((2, 128, 256))
    for c in range(2):
        tinv[c, :, 0:128] = TinvT.real[c * 128:(c + 1) * 128]
        tinv[c, :, 128:256] = TinvT.imag[c * 128:(c + 1) * 128]
    E2 = np.exp(+2j * np.pi * np.outer(k2, n2) / N2) * (2.0 / N)  # (k2, n2)
    e2 = np.zeros((2, 128, 512))
    for c in range(2):
        e2[c, :, 0:256] = E2.real[c * 128:(c + 1) * 128]
        e2[c, :, 256:512] = -E2.imag[c * 128:(c + 1) * 128]
    return W2cat, e1cat, tinv, e2


def _build_graph():
    import concourse.bass as bass
    import concourse.mybir as mybir
    import concourse.tile as tile
    from concourse import bacc

    dt = mybir.dt
    nc = bacc.Bacc("TRN2", target_bir_lowering=False, debug=False, num_devices=NCORES)

    rec_d = nc.dram_tensor("rec", [BC, E, 128, RECW], dt.bfloat16, kind="ExternalInput")
    w2_d = nc.dram_tensor("w2cat", [N2, 512], dt.bfloat16, kind="ExternalInput")
    e1_d = nc.dram_tensor("e1cat", [K1, 384], dt.bfloat16, kind="ExternalInput")
    tinv_d = nc.dram_tensor("tinv", [2, 128, 256], dt.bfloat16, kind="ExternalInput")
    e2_d = nc.dram_tensor("e2", [2, 128, 512], dt.bfloat16, kind="ExternalInput")
    out_d = nc.dram_tensor("out", [BC, N2, N1], dt.float32, kind="ExternalOutput")
    aux_d = nc.dram_tensor("aux", [BC, 2], dt.float32, kind="ExternalOutput")

    LAG = 3

    with tile.TileContext(nc) as tc:
        with (
            tc.tile_pool(name="const", bufs=1) as cpool,
            tc.tile_pool(name="rec", bufs=LAG + 3) as recpool,
            tc.tile_pool(name="work", bufs=6) as pool,
            tc.tile_pool(name="inv", bufs=2) as ipool,
            tc.tile_pool(name="p1p", bufs=3, space="PSUM") as p1pool,
            tc.tile_pool(name="pxp", bufs=2, space="PSUM") as pxpool,
            tc.tile_pool(name="pgp", bufs=2, space="PSUM") as pgpool,
            tc.tile_pool(name="pyp", bufs=1, space="PSUM") as pypool,
        ):
            w2_0 = cpool.tile([128, 512], dt.bfloat16, tag="w2_0")
            w2_1 = cpool.tile([128, 512], dt.bfloat16, tag="w2_1")
            nc.sync.dma_start(w2_0[:], w2_d[0:128, :])
            nc.sync.dma_start(w2_1[:], w2_d[128:256, :])
            e1 = cpool.tile([K1, 384], dt.bfloat16, tag="e1")
            nc.sync.dma_start(e1[:], e1_d[:])
            tinv_0 = cpool.tile([128, 256], dt.bfloat16, tag="tinv0")
            tinv_1 = cpool.tile([128, 256], dt.bfloat16, tag="tinv1")
            nc.sync.dma_start(tinv_0[:], tinv_d[0])
            nc.sync.dma_start(tinv_1[:], tinv_d[1])
            e2_0 = cpool.tile([128, 512], dt.bfloat16, tag="e2_0")
            e2_1 = cpool.tile([128, 512], dt.bfloat16, tag="e2_1")
            nc.sync.dma_start(e2_0[:], e2_d[0])
            nc.sync.dma_start(e2_1[:], e2_d[1])
            tinv = [tinv_0, tinv_1]
            e2t = [e2_0, e2_1]
            w2 = [w2_0, w2_1]

            # HAM warm-up: ~4us of back-to-back dummy matmuls un-throttle the
            # PE clock gate (4/8 -> 8/8, 1.2 -> 2.4 GHz).  The real matmul
            # stream afterwards never leaves a >3us PE-idle window, so the
            # clock stays warm for the whole kernel.
            pwarm = pypool.tile([128, 512], dt.float32, tag="pY", name="pwarm")
            for _ in range(40):
                nc.tensor.matmul(pwarm[:, 0:128], w2_0[:, 0:128],
                                 w2_0[:, 0:128], start=True, stop=True)

            slots = {}

            def front(i):
                b, e = divmod(i, E)
                rec = recpool.tile([128, RECW], dt.bfloat16, tag="rec")
                nc.sync.dma_start(rec[:], rec_d[b, e])
                p1 = p1pool.tile([128, 512], dt.float32, tag="p1")
                nc.tensor.matmul(p1[:], rec[:, 0:128], w2[0][:], start=True, stop=False)
                nc.tensor.matmul(p1[:], rec[:, 128:256], w2[1][:], start=False, stop=True)
                slots[i] = (rec, p1)

            def back(i):
                b, e = divmod(i, E)
                rec, p1 = slots.pop(i)
                p1sb = pool.tile([128, 512], dt.bfloat16, tag="p1sb")
                nc.scalar.copy(p1sb[:], p1[:])
                m12 = pool.tile([128, 1024], dt.bfloat16, tag="m12")
                uv = pool.tile([128, 512], dt.bfloat16, tag="uv")
                # m12 = [p1re*Cre | p1im*(-Cim) | p1re*Cim | p1im*Cre]:
                # one FD-1024 multiply reading p1sb twice via a 0-stride dim.
                p1rep = p1sb[:].unsqueeze(1).broadcast_to([128, 2, 512])
                nc.vector.tensor_mul(
                    m12[:].rearrange("p (a b) -> p a b", a=2),
                    p1rep, rec[:, 256:1280].rearrange("p (a b) -> p a b", a=2))
                # uv = [Ure | Uim] = pairwise block add of m12.
                m12v = m12[:].rearrange("p (a b) -> p a b", a=2)
                nc.vector.tensor_add(
                    uv[:].rearrange("p (a b) -> p a b", a=2),
                    m12v[:, :, 0:256], m12v[:, :, 256:512])
                if e == 0:
                    slots[("pX", b)] = pxpool.tile([K1, 512], dt.float32, tag="pX",
                                                   name="pX")
                pX = slots[("pX", b)]
                # One accumulation group per PSUM bank: start only on the very
                # first matmul (start marks the whole 2KB zero region), stop on
                # the very last.
                st = e == 0
                sp = e == E - 1
                nc.tensor.matmul(pX[:, 0:256], rec[:, 1280:1345], uv[:, 0:256],
                                 start=st, stop=False)
                nc.tensor.matmul(pX[:, 0:256], rec[:, 1410:1475], uv[:, 256:512],
                                 start=False, stop=False)
                nc.tensor.matmul(pX[:, 256:512], rec[:, 1280:1345], uv[:, 256:512],
                                 start=False, stop=False)
                nc.tensor.matmul(pX[:, 256:512], rec[:, 1345:1410], uv[:, 0:256],
                                 start=False, stop=sp)
                if e == E - 1:
                    inverse(i, b, slots.pop(("pX", b)))

            pending = {}

            def sched(idx, fn):
                pending.setdefault(idx, []).append(fn)

            def inverse(i, b, pX):
                # The inverse is issued in stages deferred across the next
                # events so each instruction reaches its engine queue with
                # inputs already computed (no head-of-line blocking).
                xsb = ipool.tile([K1, 512], dt.bfloat16, tag="xsb")
                pG = pgpool.tile([128, 512], dt.float32, tag="pG", name="pG")
                pY = pypool.tile([128, 512], dt.float32, tag="pY", name="pY")

                def st1():
                    nc.scalar.copy(xsb[:], pX[:])
                    nc.gpsimd.dma_start(aux_d[b, 0:1], xsb[0:1, 0:1])
                    nc.gpsimd.dma_start(aux_d[b, 1:2], xsb[64:65, 0:1])

                def st2():
                    for c in range(2):
                        xre = xsb[:, c * 128:(c + 1) * 128]
                        xim = xsb[:, 256 + c * 128:256 + (c + 1) * 128]
                        o = c * 256
                        nc.tensor.matmul(pG[:, o:o + 128], xre, e1[:, 0:128],
                                         start=(c == 0), stop=False)
                        nc.tensor.matmul(pG[:, o:o + 128], xim, e1[:, 256:384],
                                         start=False, stop=False)
                        nc.tensor.matmul(pG[:, o + 128:o + 256], xre,
                                         e1[:, 128:256], start=False, stop=False)
                        nc.tensor.matmul(pG[:, o + 128:o + 256], xim,
                                         e1[:, 0:128], start=False, stop=(c == 1))

                gts = []

                def st3():
                    for c in range(2):
                        gsb = ipool.tile([128, 256], dt.bfloat16, tag=f"gsb{c}")
                        nc.scalar.copy(gsb[:], pG[:, c * 256:(c + 1) * 256])
                        g1 = ipool.tile([128, 128], dt.bfloat16, tag=f"g1{c}")
                        g2 = ipool.tile([128, 128], dt.bfloat16, tag=f"g2{c}")
                        g3 = ipool.tile([128, 128], dt.bfloat16, tag=f"g3{c}")
                        g4 = ipool.tile([128, 128], dt.bfloat16, tag=f"g4{c}")
                        gt = ipool.tile([128, 256], dt.bfloat16, tag=f"gt{c}")
                        nc.vector.tensor_mul(g1[:], gsb[:, 0:128],
                                             tinv[c][:, 0:128])
                        nc.vector.tensor_mul(g2[:], gsb[:, 128:256],
                                             tinv[c][:, 128:256])
                        nc.vector.tensor_sub(gt[:, 0:128], g1[:], g2[:])
                        nc.gpsimd.tensor_mul(g3[:], gsb[:, 0:128],
                                             tinv[c][:, 128:256])
                        nc.gpsimd.tensor_mul(g4[:], gsb[:, 128:256],
                                             tinv[c][:, 0:128])
                        nc.vector.tensor_add(gt[:, 128:256], g3[:], g4[:])
                        gts.append(gt)

                def st4():
                    for j in range(2):
                        nc.tensor.matmul(pY[:, j * 128:(j + 1) * 128],
                                         e2t[0][:, j * 128:(j + 1) * 128],
                                         gts[0][:, 0:128], start=(j == 0),
                                         stop=False)
                        nc.tensor.matmul(pY[:, j * 128:(j + 1) * 128],
                                         e2t[0][:, 256 + j * 128:256 + (j + 1) * 128],
                                         gts[0][:, 128:256], start=False,
                                         stop=False)
                        nc.tensor.matmul(pY[:, j * 128:(j + 1) * 128],
                                         e2t[1][:, j * 128:(j + 1) * 128],
                                         gts[1][:, 0:128], start=False,
                                         stop=False)
                        nc.tensor.matmul(pY[:, j * 128:(j + 1) * 128],
                                         e2t[1][:, 256 + j * 128:256 + (j + 1) * 128],
                                         gts[1][:, 128:256], start=False,
                                         stop=(j == 1))

                def st5():
                    for j in range(2):
                        ysb = ipool.tile([128, 128], dt.float32, tag=f"ysb{j}")
                        nc.scalar.copy(ysb[:], pY[:, j * 128:(j + 1) * 128])
                        nc.sync.dma_start(out_d[b, j * 128:(j + 1) * 128, :],
                                          ysb[:])

                sched(i, st1)
                sched(i, st2)
                sched(i, st3)
                sched(i, st4)
                sched(i, st5)

            for i in range(S + LAG + 10):
                if i < S:
                    front(i)
                j = i - LAG
                if 0 <= j < S:
                    back(j)
                for fn in pending.pop(j, ()):
                    fn()

    nc.compile()
    return nc


def kernel(time_latent, stems, targets, W_pos, b_pos):
    from concourse.bass_utils import run_bass_kernel_spmd

    # host: positions (tiny linear+sigmoid, fp32 exactly like the reference)
    z = np.einsum("bed,od->beo", time_latent.astype(F32), W_pos.astype(F32))
    z = z.reshape(B, E) + b_pos.reshape(1)[0]
    pos = 1.0 / (1.0 + np.exp(-z, dtype=F32))
    s = (pos * np.float32(N)).astype(np.float64)

    W2cat, e1cat, tinv, e2 = _host_consts()
    n1 = np.arange(N1)
    k2 = np.arange(N2)
    k1 = np.arange(K1)
    T = np.exp(-2j * np.pi * np.outer(n1, k2) / N)   # (n1, k2)
    W1 = np.exp(-2j * np.pi * np.outer(n1, k1) / N1)  # (n1, k1)

    w2cat_b = W2cat.astype(BF16)
    e1cat_b = e1cat.astype(BF16)
    tinv_b = tinv.astype(BF16)
    e2_b = e2.astype(BF16)

    nc = _build_graph()
    in_maps = []
    for c in range(NCORES):
        sl = slice(c * BC, (c + 1) * BC)
        s_flat = s[sl].reshape(-1)                                   # (S,)
        rec = np.empty((S, 128, RECW), dtype=BF16)
        # stems: (S, 256, 128) -> (S, 2, 128, 128) -> (S, 128, 2, 128)
        st = stems[sl].reshape(S, 2, 128, 128).transpose(0, 2, 1, 3)
        rec[:, :, 0:256] = st.reshape(S, 128, 256).astype(BF16)
        A = np.exp(-2j * np.pi * np.outer(s_flat, k2) / N)           # (S, k2)
        C = T[None, :, :] * A[:, None, :]                            # (S, n1, k2)
        cre = C.real.astype(BF16)
        cim = C.imag.astype(BF16)
        rec[:, :, 256:512] = cre
        rec[:, :, 512:768] = -cim
        rec[:, :, 768:1024] = cim
        rec[:, :, 1024:1280] = cre
        del C, cre, cim
        Bt = np.exp(-2j * np.pi * np.outer(s_flat, k1) / N1)         # (S, k1)
        M = W1[None, :, :] * Bt[:, None, :]                          # (S, n1, k1)
        rec[:, :, 1280:1345] = M.real.astype(BF16)
        rec[:, :, 1345:1410] = M.imag.astype(BF16)
        rec[:, :, 1410:1475] = (-M.imag).astype(BF16)
        rec[:, :, 1475:1476] = 0
        del M
        in_maps.append({
            "rec": rec.reshape(BC, E, 128, RECW),
            "w2cat": w2cat_b,
            "e1cat": e1cat_b,
            "tinv": tinv_b,
            "e2": e2_b,
        })

    import os
    trace = bool(int(os.environ.get("ATHENA_TRACE", "0")))
    res = run_bass_kernel_spmd(nc, in_maps, core_ids=list(range(NCORES)), trace=trace)
    if trace:
        print(f"HW exec time: {res.exec_time_ns} ns")
    outs = []
    sign = np.where(np.arange(N) % 2 == 0, 1.0, -1.0).astype(F32)
    for c in range(NCORES):
        y = res.results[c]["out"].reshape(BC, N).astype(F32)
        aux = res.results[c]["aux"].astype(F32)          # (BC, 2) = X0, XNyq
        y = y + (-aux[:, 0:1] + sign[None, :] * aux[:, 1:2]) / np.float32(N)
        outs.append(y)
    return np.concatenate(outs, 0).reshape(B, 1, N).astype(F32)
